# revision 1
# baseline (speedup 1.0000x reference)
"""Causal self-attention (B=2, T=2048, C=2048, H=16, D=128, RoPE) on 8 trn2 cores.

Sharding (Megatron-style tensor parallel + data parallel over batch):
  core c -> batch b = c // 4, heads h in [4*(c%4), 4*(c%4)+4).
Each core computes the qkv projection for its 4 heads (c_att column-parallel),
RoPE, causal attention, and its partial row-parallel c_proj output [T, C];
the host sums the 4 partials per batch and adds the biases.

Layout trick: QKV is computed feature-major (features on partitions, tokens on
the free dim) so q@k^T and attn@v need no on-device transposes:
  scoresT[k,q] = kT.T @ qT, probsT = exp(scoresT/sqrt(D)) * causal_mask,
  denom[q] = ones.T @ probsT (PE), yT[d,q] = (v.T @ probsT) * (1/denom),
  out[tok,co] = sum_h yT_h.T @ w_projT_h.
RoPE pairs (2i, 2i+1) become (i, i+64) by permuting w_att rows on the host, so
the rotation is a partition-half swap plus two multiplies and an add on DVE.

All matmuls run in float32r (fp32 with 11-bit mantissa at full PE rate;
measured rel err ~1e-4 vs fp32).
"""
import numpy as np

import bass_rust
import concourse.bass as bass
import concourse.tile as tile
from concourse import mybir
from concourse.bass_utils import run_bass_kernel_spmd
from concourse.vector_clock import ScopedClock

B, T, C = 2, 2048, 2048
H_TOT, D = 16, 128
HPC = 4              # heads per core
N_CORES = 8
NKC = C // 128       # contraction chunks (16)
NT = T // 512        # 512-token tiles (4)
NST = T // 128       # 128-token subtiles (16)
F32R = mybir.dt.float32r
F32 = mybir.dt.float32
ROPE_BASE = 10000.0

_splitctr = [0]


class _SplitWaitTileContext(tile.TileContext):
    """This walrus build allows <=1 sync wait per instruction (<=2 for
    EventSemaphore); stock Tile can emit more on matmuls and on the tail
    drain. Hoist excess waits onto preceding same-engine NOPs."""

    def _add_instruction(self, inst):
        si = inst.sync_info
        if si is not None and si.on_wait:
            waits = list(si.on_wait)
            cap = 2 if isinstance(inst, mybir.InstEventSemaphore) else 1
            if len(waits) > cap:
                for w in waits[cap:]:
                    _splitctr[0] += 1
                    nop = mybir.InstNoOp(
                        name=f"{inst.name}-wsplit{_splitctr[0]}",
                        sync_info=mybir.SyncInfo(on_wait=[w], on_update=[]),
                        bass_nofuse=True,
                        engine=inst.engine,
                    )
                    super()._add_instruction(nop)
                si.on_wait = waits[:cap]
        super()._add_instruction(inst)

    def _drain_and_barrier(self, tick_clock, wait_clock):
        nc = self.nc
        drain_inst = nc.sync.drain()
        wait_clock.add_sem_waits(
            drain_inst.ins, ScopedClock({None: tick_clock.global_clock})
        )
        si = drain_inst.ins.sync_info
        waits = list(si.on_wait or [])
        if len(waits) > 1:
            si.on_wait = waits[:1]
            for w in waits[1:]:
                nop = nc.sync.nop(nofuse=True, hint="drain_wait_split")
                nop.ins.sync_info = bass_rust.SyncInfo(on_wait=[w], on_update=[])

        nc.all_engine_barrier()
        assert self.sems is not None
        popped = nc._tile_sem_poison_stack.pop()
        assert popped is self._sem_poison
        nc.clear_and_free_semaphores(list(self.sems.allocated().values()))
        nc.all_engine_barrier()


def _emit_phase1(nc, tc, aps, consts, acts, with_bias=True):
    """QKV projection + bias + rope; feature-major for q/k, token-major for v.

    Token tiles go in pairs so q / k / v each get a full 8-bank PSUM pass per
    pair and each weight chunk is loaded only twice. Biases enter as K=1
    matmuls (bias row x ones row). RoPE reads the PSUM tile directly: the
    half-swap is expressed as partition-offset operand slices on DVE."""
    xt, wqk, wv = aps["xt"], aps["wqk"], aps["wv"]
    cost, sins = aps["cost"], aps["sins"]
    bqkr_sb, bvr_sb = consts["bqkr_sb"], consts["bvr_sb"]
    ones_row, ones_512 = consts["ones_row"], consts["ones_512"]
    qrot, krot, v_sb = acts["qrot"], acts["krot"], acts["v_sb"]

    with (
        tc.tile_pool(name="ropec", bufs=1) as ropecp,
        tc.tile_pool(name="xc", bufs=17 if bqkr_sb is None else 16) as xcp,
        tc.tile_pool(name="wstr", bufs=4) as wstrp,
        tc.tile_pool(name="qtr", bufs=2) as qtrp,
        tc.tile_pool(name="ps1", bufs=8, space="PSUM") as ps1,
    ):
        cos_sb = ropecp.tile([128, T], F32, tag="cos")
        nc.sync.dma_start(cos_sb[:], cost[:, :])
        sin_sb = ropecp.tile([128, T], F32, tag="sin")
        nc.sync.dma_start(sin_sb[:], sins[:, :])
        for npair in range(2):
            xcw = []
            for k in range(NKC):
                t_ = xcp.tile([128, 1024], F32R, tag="xc")
                eng = nc.sync if k < 4 else nc.gpsimd
                eng.dma_start(
                    t_[:], xt[k * 128:(k + 1) * 128,
                              npair * 1024:(npair + 1) * 1024]
                )
                xcw.append(t_)
            xc = [[t[:, 0:512] for t in xcw], [t[:, 512:1024] for t in xcw]]

            # half 0: q features of the 4 heads; half 1: k features
            for half in range(2):
                pss_ = [
                    ps1.tile([128, 512], F32, tag="ps", name=f"p1qk{npair}{half}{i}")
                    for i in range(8)
                ]
                if with_bias:
                    for i in range(8):
                        m = half * 4 + i // 2
                        nc.tensor.matmul(
                            pss_[i][:], bqkr_sb[0:1, m * 128:(m + 1) * 128],
                            ones_512, start=True, stop=False,
                        )
                for k in range(NKC):
                    w_ = wstrp.tile([128, 512], F32R, tag="w", bufs=5)
                    nc.sync.dma_start(
                        w_[:],
                        wqk[k * 128:(k + 1) * 128, half * 512:(half + 1) * 512],
                    )
                    for ml in range(4):
                        for j in range(2):
                            nc.tensor.matmul(
                                pss_[ml * 2 + j][:],
                                w_[:, ml * 128:(ml + 1) * 128],
                                xc[j][k][:],
                                start=(not with_bias and k == 0),
                                stop=(k == NKC - 1),
                            )
                for ml in range(4):
                    for j in range(2):
                        n = 2 * npair + j
                        tok = slice(n * 512, (n + 1) * 512)
                        ps = pss_[ml * 2 + j]
                        dst = (qrot if half == 0 else krot)[ml]
                        nc.vector.tensor_mul(dst[:, tok], ps[:], cos_sb[:, tok])
                        m2 = qtrp.tile([128, 512], F32, tag="m2")
                        nc.vector.tensor_mul(
                            m2[0:64, :], ps[64:128, :], sin_sb[0:64, tok])
                        nc.vector.tensor_mul(
                            m2[64:128, :], ps[0:64, :], sin_sb[64:128, tok])
                        nc.vector.tensor_add(dst[:, tok], dst[:, tok], m2[:])

            # v pass: token-major
            pssv = [
                ps1.tile([128, 512], F32, tag="ps", name=f"p1v{npair}{i}")
                for i in range(8)
            ]
            if with_bias:
                for i in range(8):
                    nc.tensor.matmul(
                        pssv[i][:], ones_row, bvr_sb[:], start=True, stop=False,
                    )
            for k in range(NKC):
                wv_ = wstrp.tile([128, HPC * D], F32R, tag="wv2", bufs=4)
                nc.sync.dma_start(wv_[:], wv[k * 128:(k + 1) * 128, :])
                for j in range(2):
                    for s in range(4):
                        nc.tensor.matmul(
                            pssv[j * 4 + s][:],
                            xc[j][k][:, s * 128:(s + 1) * 128],
                            wv_[:],
                            start=(not with_bias and k == 0),
                            stop=(k == NKC - 1),
                        )
            for j in range(2):
                for s in range(4):
                    nc.scalar.copy(
                        v_sb[4 * (2 * npair + j) + s][:], pssv[j * 4 + s][:]
                    )


def _emit_phase2(nc, tc, aps, consts, acts):
    """Causal attention in the transposed layout; softmax denominators via
    ones-matmul on PE, normalization fused into the psum->sbuf drain."""
    masks = aps["masks"]
    ones_sb, ones_row = consts["ones_sb"], consts["ones_row"]
    qrot, krot, v_sb, y_sb = acts["qrot"], acts["krot"], acts["v_sb"], acts["y_sb"]
    scale = 1.0 / float(np.sqrt(D))

    with (
        tc.tile_pool(name="maskp", bufs=1) as maskp,
        tc.tile_pool(name="probs", bufs=9) as probsp,
        tc.tile_pool(name="dn", bufs=3) as dnp,
        tc.tile_pool(name="pss", bufs=4, space="PSUM") as pss,
        tc.tile_pool(name="psy", bufs=2, space="PSUM") as psy,
        tc.tile_pool(name="psd", bufs=1, space="PSUM") as psd,
        tc.tile_pool(name="psb", bufs=1, space="PSUM") as psb,
    ):
        mask_sb = [
            maskp.tile([128, 512], F32R, tag=f"mask{j}", name=f"mask{j}")
            for j in range(4)
        ]
        for j in range(4):
            nc.sync.dma_start(mask_sb[j][:], masks[j, :, :])

        for h in range(HPC):
            for t in range(NT):
                qsl = slice(t * 512, (t + 1) * 512)
                nch = 4 * (t + 1)
                ps_y = psy.tile([128, 512], F32, tag="ps", name=f"psy{h}{t}")
                ps_d = psd.tile([1, 512], F32, tag="ps", name=f"psd{h}{t}")
                for ci in range(nch):
                    j = ci - 4 * t
                    # columns q < 128j of a diagonal chunk are fully masked:
                    # clip them out (keep N >= 256 for the fp32r fast path)
                    qo = 0 if (j < 1 or ci == 0) else min(128 * j, 256)
                    csl = slice(qo, 512)
                    ps_s = pss.tile([128, 512], F32, tag="ps", name=f"pss{h}{t}{ci}")
                    nc.tensor.matmul(
                        ps_s[:, csl], krot[h][:, ci * 128:(ci + 1) * 128],
                        qrot[h][:, t * 512 + qo:(t + 1) * 512],
                        start=True, stop=True,
                    )
                    pr = probsp.tile([128, 512], F32R, tag="pr")
                    nc.scalar.activation(
                        pr[:, csl], ps_s[:, csl],
                        mybir.ActivationFunctionType.Exp, scale=scale,
                    )
                    if j >= 0:
                        nc.vector.tensor_mul(pr[:, csl], pr[:, csl],
                                             mask_sb[j][:, csl])
                    nc.tensor.matmul(
                        ps_d[0:1, csl], ones_sb, pr[:, csl],
                        start=(ci == 0), stop=(ci == nch - 1),
                    )
                    nc.tensor.matmul(
                        ps_y[:, csl], v_sb[ci][:, h * 128:(h + 1) * 128],
                        pr[:, csl],
                        start=(ci == 0), stop=(ci == nch - 1),
                    )
                den = dnp.tile([1, 512], F32R, tag="den")
                nc.scalar.copy(den[:], ps_d[:])
                ps_b = psb.tile([128, 512], F32, tag="ps", name=f"psb{h}{t}")
                nc.tensor.matmul(ps_b[:], ones_row, den[:], start=True, stop=True)
                bc = dnp.tile([128, 512], F32, tag="bc")
                nc.vector.reciprocal(bc[:], ps_b[:])
                nc.vector.tensor_mul(y_sb[h][:, qsl], ps_y[:], bc[:])


def _emit_phase3(nc, tc, aps, consts, acts, wp_sb):
    """Row-parallel projection partial: out[tok, co] = sum_h yT_h.T @ wpT_h."""
    out = aps["out"]
    y_sb = acts["y_sb"]

    with (
        tc.tile_pool(name="outp", bufs=4) as outp,
        tc.tile_pool(name="pso", bufs=8, space="PSUM") as pso,
    ):
        for tt in range(NST):
            tsl = slice(tt * 128, (tt + 1) * 128)
            o_sb = outp.tile([128, C], F32, tag="osb")
            for co in range(4):
                ps = pso.tile([128, 512], F32, tag="ps", name=f"pso{tt}{co}")
                for h in range(HPC):
                    nc.tensor.matmul(
                        ps[:], y_sb[h][:, tsl],
                        wp_sb[h][:, co * 512:(co + 1) * 512],
                        start=(h == 0), stop=(h == HPC - 1),
                    )
                nc.scalar.copy(o_sb[:, co * 512:(co + 1) * 512], ps[:])
                nc.sync.dma_start(
                    out[tsl, co * 512:(co + 1) * 512],
                    o_sb[:, co * 512:(co + 1) * 512])


def _build_program(phases=(1, 2, 3), reps=1, with_bias=True):
    nc = bass.Bass("TRN2", target_bir_lowering=False, debug=False)

    aps = {
        "xt": nc.dram_tensor("xt", [C, T], F32R, kind="ExternalInput").ap(),
        "wqk": nc.dram_tensor("wqk", [C, 2 * HPC * D], F32R, kind="ExternalInput").ap(),
        "wv": nc.dram_tensor("wv", [C, HPC * D], F32R, kind="ExternalInput").ap(),
        "wp": nc.dram_tensor("wp", [HPC * D, C], F32R, kind="ExternalInput").ap(),
        "cost": nc.dram_tensor("cost", [128, T], F32, kind="ExternalInput").ap(),
        "sins": nc.dram_tensor("sins", [128, T], F32, kind="ExternalInput").ap(),
        "masks": nc.dram_tensor("masks", [4, 128, 512], F32R, kind="ExternalInput").ap(),
        "bqkr": nc.dram_tensor("bqkr", [1, 2 * HPC * 128], F32R, kind="ExternalInput").ap(),
        "bvr": nc.dram_tensor("bvr", [1, HPC * D], F32R, kind="ExternalInput").ap(),
        "onesd": nc.dram_tensor("onesd", [128, 512], F32R, kind="ExternalInput").ap(),
        "out": nc.dram_tensor("out", [T, C], F32, kind="ExternalOutput").ap(),
    }

    with _SplitWaitTileContext(nc) as tc:
      for _rep in range(reps):
          if _rep:
              tc.strict_bb_all_engine_barrier()
          with (
              tc.tile_pool(name="const", bufs=1) as constp,
              tc.tile_pool(name="qkrot", bufs=1) as qkrotp,
              tc.tile_pool(name="vsb", bufs=1) as vp,
          ):
              if with_bias:
                  bqkr_sb = constp.tile([1, 2 * HPC * 128], F32R, tag="bqkr")
                  nc.sync.dma_start(bqkr_sb[:], aps["bqkr"][:, :])
                  bvr_sb = constp.tile([1, HPC * D], F32R, tag="bvr")
                  nc.sync.dma_start(bvr_sb[:], aps["bvr"][:, :])
              else:
                  bqkr_sb = bvr_sb = None
              ones_t = constp.tile([128, 512], F32R, tag="ones")
              nc.sync.dma_start(ones_t[:], aps["onesd"][:, :])
              consts = {
                  "bqkr_sb": bqkr_sb,
                  "bvr_sb": bvr_sb,
                  "ones_sb": ones_t[:, 0:1],
                  "ones_row": ones_t[0:1, 0:128],
                  "ones_512": ones_t[0:1, :],
              }
              acts = {
                  "qrot": [qkrotp.tile([128, T], F32R, tag=f"qrot{h}", name=f"qrot{h}")
                           for h in range(HPC)],
                  "krot": [qkrotp.tile([128, T], F32R, tag=f"krot{h}", name=f"krot{h}")
                           for h in range(HPC)],
                  "v_sb": [vp.tile([128, HPC * D], F32R, tag=f"v{s}", name=f"v{s}")
                           for s in range(NST)],
              }

              if 1 in phases:
                  _emit_phase1(nc, tc, aps, consts, acts, with_bias=with_bias)

              with (
                  tc.tile_pool(name="ysb", bufs=1) as yp,
                  tc.tile_pool(name="wpp", bufs=1) as wpp,
              ):
                  acts["y_sb"] = [
                      yp.tile([128, T], F32R, tag=f"y{h}", name=f"y{h}")
                      for h in range(HPC)
                  ]
                  wp_sb = [
                      wpp.tile([128, C], F32R, tag=f"wp{h}", name=f"wp{h}")
                      for h in range(HPC)
                  ]
                  for h in range(HPC):
                      nc.sync.dma_start(
                          wp_sb[h][:], aps["wp"][h * 128:(h + 1) * 128, :])
                  if 2 in phases:
                      _emit_phase2(nc, tc, aps, consts, acts)
                  if 3 in phases:
                      _emit_phase3(nc, tc, aps, consts, acts, wp_sb)


    return nc


_prog_cache = {}


def _get_program(with_bias=True):
    key = f"nc{with_bias}"
    if key not in _prog_cache:
        _prog_cache[key] = _build_program(with_bias=with_bias)
    return _prog_cache[key]


def _host_prep(x, w_att, b_att, w_proj):
    """Build the 8 per-core input maps."""
    perm = np.concatenate([np.arange(0, 128, 2), np.arange(1, 128, 2)])

    # rope tables in the permuted layout: rows (i, i+64) pair with angle theta_i
    theta = 1.0 / (ROPE_BASE ** (np.arange(0, D, 2, dtype=np.float64) / D))  # [64]
    freqs = np.arange(T, dtype=np.float64)[:, None] * theta[None, :]  # [T, 64]
    cos = np.cos(freqs).astype(np.float32).T  # [64, T]
    sin = np.sin(freqs).astype(np.float32).T
    cost = np.concatenate([cos, cos], axis=0)          # [128, T]
    sins = np.concatenate([-sin, sin], axis=0)         # [128, T]

    # causal masks for the 4 diagonal chunk offsets
    kk = np.arange(128)[:, None]
    qq = np.arange(512)[None, :]
    masks = np.stack(
        [(128 * j + kk <= qq).astype(np.float32) for j in range(4)]
    )  # [4,128,512]

    in_maps = []
    for c in range(N_CORES):
        b = c // 4
        h0 = HPC * (c % 4)
        xtb = np.ascontiguousarray(x[b].T)  # [C, T]

        wq_cols, wk_cols, bq, bk = [], [], [], []
        for h in range(h0, h0 + HPC):
            wq_cols.append(w_att[h * D + perm, :].T)            # [C,128]
            wk_cols.append(w_att[C + h * D + perm, :].T)
            bq.append(b_att[h * D + perm])
            bk.append(b_att[C + h * D + perm])
        wqk = np.ascontiguousarray(
            np.concatenate(wq_cols + wk_cols, axis=1)
        )  # [C, 1024]
        bqkr = np.concatenate(bq + bk).astype(np.float32).reshape(1, -1)  # [1, 1024]
        bvr = np.ascontiguousarray(
            b_att[2 * C + h0 * D:2 * C + (h0 + HPC) * D]
        ).astype(np.float32).reshape(1, HPC * D)

        wv_ = np.ascontiguousarray(
            w_att[2 * C + h0 * D:2 * C + (h0 + HPC) * D, :].T
        )  # [C, 512]
        wp_ = np.ascontiguousarray(
            w_proj[:, h0 * D:(h0 + HPC) * D].T
        )  # [512, C]

        in_maps.append({
            "xt": xtb, "wqk": wqk, "wv": wv_, "wp": wp_,
            "cost": cost, "sins": sins, "masks": masks,
            "bqkr": bqkr, "bvr": bvr,
            "onesd": np.ones((128, 512), np.float32),
        })
    return in_maps


def kernel(x, w_att, b_att, w_proj, b_proj):
    x = np.asarray(x, dtype=np.float32)
    w_att = np.asarray(w_att, dtype=np.float32)
    b_att = np.asarray(b_att, dtype=np.float32)
    w_proj = np.asarray(w_proj, dtype=np.float32)
    b_proj = np.asarray(b_proj, dtype=np.float32)

    nc = _get_program(with_bias=bool(np.any(b_att)))
    in_maps = _host_prep(x, w_att, b_att, w_proj)
    res = run_bass_kernel_spmd(nc, in_maps, list(range(N_CORES)))

    bias = b_proj
    out = np.empty((B, T, C), dtype=np.float32)
    for b in range(B):
        acc = res.results[4 * b]["out"].astype(np.float64)
        for g in range(1, 4):
            acc = acc + res.results[4 * b + g]["out"]
        out[b] = (acc + bias).astype(np.float32)
    return out

